# revision 2
# baseline (speedup 1.0000x reference)
"""Trainium2 Bass kernel for nn_ConvBlock (MuLUT-style conv block), v4.

Sharding: 8 cores = 8 (rotation, batch) pairs; each core computes all 6
branches over its 4096 rotated+padded pixels; host does im2col prep and the
B6/tanh/round/shuffle/unrotate/sum epilogue.

v4 (vs v3 baseline, 146.8us):
  - Conv streams M=128-packed across branch pairs sharing the same input
    taps: pairs (0,1) K=18, (2,5) K=50, (3,4) K=18 per chunk-pair, one
    stream each instead of one per branch (-24 matmuls, -24 LDWs).
    XC layout: ch0 taps rows 0:18, ch1 taps rows 32:50 (32-aligned so
    tile_position works for the (3,4) pair).
  - s5 (w4 h2-part) runs K=64 on h23[0:64] directly instead of K=128 with
    stale zero-weighted rows: drops the 6 serialized gpsimd memsets that
    gated early s5 streams.
  - Input DM487s issued from two queues (sync + scalar) and consolidated:
    scalar: WCV, BV; sync: XC cols 0:1024, WDE, WDL, XC cols 1024:4096.
    First conv only waits on WCV + XC cp0 (subtile deps).
  - PE warm-up: 5 dummy N=512 matmuls on a memset scratch tile emitted
    before real work. The HAM clock gate needs ~3.4us of sustained PE
    activity to lift the 1.2->2.4GHz throttle; the baseline spent the
    first 24us cold because the PE idled during DMA fill.
"""

import sys

import numpy as np
import ml_dtypes

if "/opt/trn_rl_repo" not in sys.path:
    sys.path.insert(0, "/opt/trn_rl_repo")

IN_C, OUT_C, SCALE, S, NF = 2, 2, 2, 3, 64
MODES = 3
NB = IN_C * MODES
PAD = S - 1
B, H = 2, 64
NPIX = H * H
NCP = 4                 # chunk-pairs per image
CPW = 1024              # pixels per chunk-pair
CW = 512                # pixels per chunk (psum bank limit in fp32)
N_CORES = 8
LANES = 6
BF16 = ml_dtypes.bfloat16

# branch order within a chunk-pair: pairs (0,1), (2,5), (3,4) adjacent
BR_ORDER = [0, 1, 2, 5, 3, 4]
# conv rhs row-range per pair index (rows of XC)
PAIR_ROWS = [(0, 18), (0, 50), (32, 50)]

_BASS_CACHE = {}


def _build_bass():
    import concourse.bass as bass  # noqa: F401
    import concourse.mybir as mybir
    from concourse import bacc
    from concourse.tile import TileContext

    f32 = mybir.dt.float32
    bf = mybir.dt.bfloat16
    Alu = mybir.AluOpType
    Act = mybir.ActivationFunctionType

    nc = bacc.Bacc(
        "TRN2",
        target_bir_lowering=False,
        debug=False,
        enable_asserts=False,
        num_devices=N_CORES,
    )

    xcol_d = nc.dram_tensor("xcol", [50, NPIX], bf, kind="ExternalInput")
    wcv_d = nc.dram_tensor("wcv", [50, 3 * 128], bf, kind="ExternalInput")
    wde_d = nc.dram_tensor("wde", [128, NB * 256], bf, kind="ExternalInput")
    wdl_d = nc.dram_tensor("wdl", [128, NB * 152], bf, kind="ExternalInput")
    bv_d = nc.dram_tensor("bvec", [64, NB * 5], f32, kind="ExternalInput")
    yout_d = nc.dram_tensor("yout", [NB, 8, NPIX], bf, kind="ExternalOutput")

    with TileContext(nc) as tc:
        with (
            tc.tile_pool(name="const", bufs=1) as cpool,
            tc.tile_pool(name="psum", bufs=3, space="PSUM") as ppool,
        ):
            XC = cpool.tile([64, NPIX], bf, name="XC")
            WCV = cpool.tile([64, 3 * 128], bf, name="WCV")
            WDE = cpool.tile([128, NB * 256], bf, name="WDE")
            WDL = cpool.tile([128, NB * 152], bf, name="WDL")
            BV = cpool.tile([64, NB * 5], f32, name="BV")
            DW = cpool.tile([128, 512], bf, name="DW")

            # Input DMAs on two issue queues. Scalar (Activation) is idle at
            # start; it brings in the small tensors the first ops need.
            nc.scalar.dma_start(out=WCV[0:50, :], in_=wcv_d.ap())
            nc.scalar.dma_start(out=BV[:, :], in_=bv_d.ap())
            _sl = slice(0, CPW)
            nc.sync.dma_start(out=XC[0:50, _sl], in_=xcol_d.ap()[:, _sl])
            nc.sync.dma_start(out=WDE[:, :], in_=wde_d.ap())
            nc.sync.dma_start(out=WDL[:, :], in_=wdl_d.ap())
            _sl = slice(CPW, NPIX)
            nc.sync.dma_start(out=XC[0:50, _sl], in_=xcol_d.ap()[:, _sl])

            # PE warm-up: memset a scratch tile on gpsimd, then issue dummy
            # matmuls so the HAM activity monitor lifts the cold-clock
            # throttle while the weight DMAs are still landing.
            nc.gpsimd.memset(DW[:, :], 0.0)
            wt = ppool.tile([128, CPW], f32, name="warm", tag="x")
            for _ in range(5):
                nc.tensor.matmul(wt[:, 0:CW], lhsT=DW[:, 0:128],
                                 rhs=DW[:, 0:CW], start=True, stop=True)

            # Fixed per-lane activation tiles (reused across units; WAR deps
            # tracked by the tile framework). Free dim holds the chunk-pair:
            # [:, 0:512] = even chunk, [:, 512:1024] = odd chunk.
            lanes = []
            for i in range(LANES):
                h01 = cpool.tile([128, CPW], bf, name=f"h01L{i}")
                h23 = cpool.tile([128, CPW], bf, name=f"h23L{i}")
                h4y = cpool.tile([128, CPW], bf, name=f"h4yL{i}")
                lanes.append((h01, h23, h4y))

            ecnt = [0]

            def act_engine():
                c = ecnt[0]
                ecnt[0] += 1
                return (c % 11) % 2 == 0  # 6 of every 11 ops on ACT

            def relu1(out_ap, in_ap, bias_ap):
                if act_engine():
                    nc.scalar.activation(out_ap, in_ap, Act.Relu,
                                         bias=bias_ap, scale=1.0)
                else:
                    nc.vector.tensor_scalar(
                        out=out_ap, in0=in_ap, scalar1=bias_ap, scalar2=0.0,
                        op0=Alu.add, op1=Alu.max)

            def relu_op(out_ap, in_ap, bias_ap, split=False):
                if split:
                    # tail units: halve the latency per chain level so the
                    # pipeline drain is shallower
                    relu1(out_ap[:, 0:CW], in_ap[:, 0:CW], bias_ap)
                    relu1(out_ap[:, CW:CPW], in_ap[:, CW:CPW], bias_ap)
                else:
                    relu1(out_ap, in_ap, bias_ap)

            def copy1(out_ap, in_ap):
                if act_engine():
                    nc.scalar.copy(out_ap, in_ap)
                else:
                    nc.vector.tensor_copy(out_ap, in_ap)

            def copy_op(out_ap, in_ap, split=False):
                if split:
                    copy1(out_ap[:, 0:CW], in_ap[:, 0:CW])
                    copy1(out_ap[:, CW:CPW], in_ap[:, CW:CPW])
                else:
                    copy1(out_ap, in_ap)

            units = [(br, cp) for cp in range(NCP) for br in BR_ORDER]
            NU = len(units)
            state = [dict() for _ in range(NU)]
            EH, OH = slice(0, CW), slice(CW, CPW)

            def wde(br, c0, c1):
                return WDE[:, br * 256 + c0: br * 256 + c1]

            def wdl(br, c0, c1):
                return WDL[:, br * 152 + c0: br * 152 + c1]

            def phase(u, ph):
                br, cp = units[u]
                tail = u >= NU - 3
                st = state[u]
                h01, h23, h4y = lanes[u % LANES]
                px = [slice(cp * CPW + k * CW, cp * CPW + (k + 1) * CW)
                      for k in range(2)]
                halves = (EH, OH)

                def bias(l):
                    j = br * 5 + l
                    return BV[:, j:j + 1]

                if ph == 0:
                    pos = u % 6
                    if pos % 2 == 0:
                        # pair-even unit: one M=128 conv stream computes the
                        # pre-activations for BOTH branches of the pair
                        pi = pos // 2
                        r0, r1 = PAIR_ROWS[pi]
                        o1 = ppool.tile([128, CPW], f32, name=f"o1_{u}",
                                        tag="x")
                        st["o1"] = o1
                        state[u + 1]["o1"] = o1
                        lhs = WCV[r0:r1, pi * 128:(pi + 1) * 128]
                        for k in range(2):
                            nc.tensor.matmul(o1[0:128, halves[k]], lhsT=lhs,
                                             rhs=XC[r0:r1, px[k]],
                                             start=True, stop=True)
                        relu_op(h01[0:64, :], o1[0:64, :], bias(0),
                                split=tail)
                    else:
                        o1 = st["o1"]
                        relu_op(h01[0:64, :], o1[64:128, :], bias(0),
                                split=tail)
                elif ph == 1:
                    a1 = ppool.tile([128, CPW], f32, name=f"a1_{u}", tag="x")
                    st["a1"] = a1
                    lhs = wde(br, 0, 64)[0:64, :]
                    for k in range(2):
                        nc.tensor.matmul(a1[0:64, halves[k]], lhsT=lhs,
                                         rhs=h01[0:64, halves[k]],
                                         start=True, stop=True)
                    relu_op(h01[64:128, :], a1[0:64, :], bias(1), split=tail)
                elif ph == 2:
                    o2 = ppool.tile([128, CPW], f32, name=f"o2_{u}", tag="x")
                    a2 = ppool.tile([128, CPW], f32, name=f"a2_{u}", tag="x")
                    st["o2"], st["a2"] = o2, a2
                    for k in range(2):
                        nc.tensor.matmul(o2[0:64, halves[k]],
                                         lhsT=wde(br, 64, 128),
                                         rhs=h01[:, halves[k]],
                                         start=True, stop=True)
                    for k in range(2):
                        nc.tensor.matmul(a2[0:64, halves[k]],
                                         lhsT=wde(br, 128, 192),
                                         rhs=h01[:, halves[k]],
                                         start=True, stop=False)
                    relu_op(h23[0:64, :], o2[0:64, :], bias(2), split=tail)
                elif ph == 3:
                    a2 = st["a2"]
                    for k in range(2):
                        nc.tensor.matmul(a2[0:64, halves[k]],
                                         lhsT=wde(br, 192, 256)[0:64, :],
                                         rhs=h23[0:64, halves[k]],
                                         start=False, stop=True)
                    relu_op(h23[64:128, :], a2[0:64, :], bias(3), split=tail)
                elif ph == 4:
                    dd = ppool.tile([128, CPW], f32, name=f"dd_{u}", tag="y",
                                    bufs=1)
                    st["dd"] = dd
                    for k in range(2):
                        nc.tensor.matmul(dd[0:72, halves[k]],
                                         lhsT=wdl(br, 0, 72),
                                         rhs=h01[:, halves[k]],
                                         start=True, stop=False)
                    for k in range(2):
                        nc.tensor.matmul(dd[0:72, halves[k]],
                                         lhsT=wdl(br, 72, 144),
                                         rhs=h23[:, halves[k]],
                                         start=False, stop=True)
                    relu_op(h4y[0:64, :], dd[0:64, :], bias(4), split=tail)
                else:
                    dd = st["dd"]
                    for k in range(2):
                        nc.tensor.matmul(dd[64:72, halves[k]],
                                         lhsT=wdl(br, 144, 152)[0:64, :],
                                         rhs=h4y[0:64, halves[k]],
                                         start=False, stop=True,
                                         skip_group_check=True)
                    copy_op(h4y[64:72, :], dd[64:72, :], split=tail)
                    nc.sync.dma_start(
                        out=yout_d.ap()[br, :, cp * CPW:(cp + 1) * CPW],
                        in_=h4y[64:72, :])

            NPH = 6
            for step in range(NU + NPH - 1):
                for ph in reversed(range(NPH)):
                    u = step - ph
                    if 0 <= u < NU:
                        phase(u, ph)

    nc.compile()
    return nc


def _get_bass():
    if "nc" not in _BASS_CACHE:
        _BASS_CACHE["nc"] = _build_bass()
    return _BASS_CACHE["nc"]


def _im2col_core(x, prev_x, r, b):
    """Per-core input: [50, 4096] bf16; ch0 taps rows 0:18, ch1 rows 32:50.

    Tap row order within a channel: ci*9 + dy*3 + dx, ci = (x, prev_x).
    """
    cols = np.zeros((50, NPIX), np.float32)
    for c in range(2):
        r0 = 0 if c == 0 else 32
        xc = np.stack([x[b, c], prev_x[b, c]])  # [2, 64, 64]
        rot = np.rot90(xc, k=r, axes=(1, 2))
        padd = np.pad(rot, ((0, 0), (0, PAD), (0, PAD)), mode="edge")
        for ci in range(2):
            for dy in range(3):
                for dx in range(3):
                    cols[r0 + ci * 9 + dy * 3 + dx] = padd[
                        ci, dy: dy + H, dx: dx + H
                    ].reshape(-1)
    return cols.astype(BF16)


def _prep_weights(W1, B1, W2, B2, W3, B3, W4, B4, W5, B5, W6, B6):
    # Conv weights packed for the three M=128 pair streams.
    # Pair pi covers branches PAIRS[pi]; member 0 -> out cols 0:64 with its
    # taps at that branch's channel row offset, member 1 -> cols 64:128.
    PAIRS = [(0, 1), (2, 5), (3, 4)]
    wcv = np.zeros((50, 3 * 128), np.float32)
    for pi, (ba, bb) in enumerate(PAIRS):
        for m, br in enumerate((ba, bb)):
            r0 = 0 if br < MODES else 32
            wcv[r0:r0 + 18, pi * 128 + m * 64: pi * 128 + (m + 1) * 64] = (
                W1[br].transpose(1, 2, 3, 0).reshape(18, 64))

    wde = np.zeros((128, NB * 256), np.float32)
    wdl = np.zeros((128, NB * 152), np.float32)
    for br in range(NB):
        be, bl = br * 256, br * 152
        w2t = W2[br].T           # [64, 64]
        w3t = W3[br].T           # [128, 64]
        w4t = W4[br].T           # [192, 64]
        w5t = W5[br].T           # [256, 64]
        w6t = W6[br].T           # [320, 8]
        wde[0:64, be + 0: be + 64] = w2t
        wde[0:128, be + 64: be + 128] = w3t
        wde[0:128, be + 128: be + 192] = w4t[0:128]
        wde[0:64, be + 192: be + 256] = w4t[128:192]
        wdl[0:128, bl + 0: bl + 64] = w5t[0:128]
        wdl[0:128, bl + 64: bl + 72] = w6t[0:128]
        wdl[0:128, bl + 72: bl + 136] = w5t[128:256]
        wdl[0:128, bl + 136: bl + 144] = w6t[128:256]
        wdl[0:64, bl + 144: bl + 152] = w6t[256:320]

    bvec = np.zeros((64, NB * 5), np.float32)
    for br in range(NB):
        for j, bb in enumerate((B1, B2, B3, B4, B5)):
            bvec[:, br * 5 + j] = bb[br]
    return wcv.astype(BF16), wde.astype(BF16), wdl.astype(BF16), bvec


def _postprocess(y_per_core, B6):
    """y_per_core[core] = yout [6, 8, 4096] bf16 (pre-B6); core = r*2 + b."""
    out = np.zeros((B, OUT_C, SCALE * H, SCALE * H), np.float32)
    for core in range(N_CORES):
        r, b = core // B, core % B
        y6 = np.asarray(y_per_core[core]).astype(np.float32) + B6[:, :, None]
        y6 = y6.reshape(NB, 8, H, H)
        z = np.round(np.tanh(y6) * np.float32(127.0))
        zz = (
            z.reshape(NB, OUT_C, SCALE, SCALE, H, H)
            .transpose(0, 1, 4, 2, 5, 3)
            .reshape(NB, OUT_C, SCALE * H, SCALE * H)
        )
        un = np.rot90(zz, k=(4 - r) % 4, axes=(2, 3))
        out[b] += un.sum(axis=0, dtype=np.float32)
    out /= np.float32(IN_C)
    return out


def kernel(x, prev_x, W1, B1, W2, B2, W3, B3, W4, B4, W5, B5, W6, B6,
           _trace=False):
    from concourse.bass_utils import run_bass_kernel_spmd

    args = [np.ascontiguousarray(np.asarray(a), dtype=np.float32) for a in
            (x, prev_x, W1, B1, W2, B2, W3, B3, W4, B4, W5, B5, W6, B6)]
    x, prev_x, W1, B1, W2, B2, W3, B3, W4, B4, W5, B5, W6, B6 = args

    wcv, wde, wdl, bvec = _prep_weights(W1, B1, W2, B2, W3, B3, W4, B4, W5,
                                        B5, W6, B6)

    in_maps = []
    for core in range(N_CORES):
        r, b = core // B, core % B
        in_maps.append(
            {
                "xcol": _im2col_core(x, prev_x, r, b),
                "wcv": wcv,
                "wde": wde,
                "wdl": wdl,
                "bvec": bvec,
            }
        )

    nc = _get_bass()
    if _trace:
        # Warmup execution: the device DVFS state alternates between runs;
        # a throwaway run first makes the traced run's clock state
        # reproducible.
        run_bass_kernel_spmd(nc, in_maps, core_ids=list(range(N_CORES)),
                             trace=False)
    res = run_bass_kernel_spmd(
        nc, in_maps, core_ids=list(range(N_CORES)), trace=_trace
    )
    _BASS_CACHE["last_results"] = res
    return _postprocess([res.results[c]["yout"] for c in range(N_CORES)], B6)
